# revision 1
# baseline (speedup 1.0000x reference)
"""Trainium2 Bass kernel for causal self-attention (RoPE + per-head RMSNorm).

Reference computation (B=2, T=2048, C=1024, H=16, D=64):
    q = rope(rmsnorm(x @ Wq.T)); k = rope(rmsnorm(x @ Wk.T)); v = x @ Wv.T
    out = softmax(causal(q k^T / sqrt(D))) v @ Wo.T

Sharding over 8 NeuronCores: core c -> batch b = c//4, head-group g = c%4
(4 heads = 256 features per group).  Everything on-chip is computed in a
feature-major ("transposed") layout so no PE transposes are needed:
  - scores are computed as S^T[tk, tq] tiles, softmax runs over the
    partition axis using matmul-with-ones tricks (denominator comes from a
    ones column appended to V), and the final division is applied via a
    K=1 broadcast matmul.
  - attention output Y^T (feature-major) is exchanged with an AllToAll
    within each batch's 4-core group, giving each core the full 1024
    features for its 512-token slice; o_proj is computed on that slice.
Host side: shards/transposes inputs (bf16), assembles the fp32 output.
"""

import os
import sys

for _p in ("/opt/trn_rl_repo", "/root/.axon_site/_ro/trn_rl_repo"):
    if os.path.isdir(_p) and _p not in sys.path:
        sys.path.insert(0, _p)

import numpy as np
import ml_dtypes

import concourse.bass as bass
from concourse import bacc
import concourse.tile as tile
import concourse.mybir as mybir

BF16 = mybir.dt.bfloat16
F32 = mybir.dt.float32
AF = mybir.ActivationFunctionType

B, T, C, H, D = 2, 2048, 1024, 16, 64
N_CORES = 8
GH = 4  # heads per core
GF = GH * D  # features per core (256)
TB = 512  # token block (matmul N)
KT = C // 128  # 8 contraction k-tiles
EPS = float(np.finfo(np.float32).eps)
ROPE_BASE = 10000.0


def build_nc(t=T):
    ntb = t // TB  # tq blocks
    ntt = t // 128  # token 128-tiles
    tsl = t // 4  # per-core token slice for o_proj

    nc = bacc.Bacc("TRN2", target_bir_lowering=False, debug=False, num_devices=N_CORES)

    xt = nc.dram_tensor("xt", [C, t], BF16, kind="ExternalInput")
    wq = nc.dram_tensor("wq", [C, GF], BF16, kind="ExternalInput")
    wk = nc.dram_tensor("wk", [C, GF], BF16, kind="ExternalInput")
    wv = nc.dram_tensor("wv", [C, GF], BF16, kind="ExternalInput")
    wo = nc.dram_tensor("wo", [2 * C, C], BF16, kind="ExternalInput")
    cosf = nc.dram_tensor("cosf", [128, t], BF16, kind="ExternalInput")
    sinf = nc.dram_tensor("sinf", [128, t], BF16, kind="ExternalInput")
    pswap = nc.dram_tensor("pswap", [128, 128], BF16, kind="ExternalInput")
    blk2 = nc.dram_tensor("blk2", [128, 2], BF16, kind="ExternalInput")
    eqb = nc.dram_tensor("eqb", [2, 128], BF16, kind="ExternalInput")
    ekb = nc.dram_tensor("ekb", [2, 128], BF16, kind="ExternalInput")
    maskt = nc.dram_tensor("maskt", [128, 4 * TB], BF16, kind="ExternalInput")
    out = nc.dram_tensor("out", [C, tsl], F32, kind="ExternalOutput")

    with tile.TileContext(nc) as tc:
        with (
            nc.allow_low_precision(reason="bf16 compute by design"),
            tc.tile_pool(name="p_xt", bufs=KT) as p_xt,
            tc.tile_pool(name="p_w", bufs=KT) as p_w,
            tc.tile_pool(name="p_wo", bufs=2 * KT) as p_wo,
            tc.tile_pool(name="p_tab", bufs=1) as p_tab,
            tc.tile_pool(name="p_qk", bufs=2) as p_qk,
            tc.tile_pool(name="p_v", bufs=ntt) as p_v,
            tc.tile_pool(name="p_y", bufs=2) as p_y,
            tc.tile_pool(name="p_yg", bufs=2 * KT) as p_yg,
            tc.tile_pool(name="p_pt", bufs=20) as p_pt,
            tc.tile_pool(name="p_tmp", bufs=2) as p_tmp,
            tc.tile_pool(name="p_mm", bufs=4, space="PSUM") as p_mm,
            tc.tile_pool(name="p_oacc", bufs=4, space="PSUM") as p_oacc,
            tc.tile_pool(name="p_dram", bufs=2, space="DRAM") as p_dram,
        ):
            # ---- load inputs -------------------------------------------------
            xt_sb = []
            wq_sb = []
            wk_sb = []
            wv_sb = []
            for ct in range(KT):
                w_t = p_w.tile([128, GF], BF16, tag="wq")
                nc.sync.dma_start(w_t[:], wq[ct * 128 : (ct + 1) * 128, :])
                wq_sb.append(w_t)
                x_t = p_xt.tile([128, t], BF16, tag="xt")
                nc.sync.dma_start(x_t[:], xt[ct * 128 : (ct + 1) * 128, :])
                xt_sb.append(x_t)
            for ct in range(KT):
                w_t = p_w.tile([128, GF], BF16, tag="wk")
                nc.sync.dma_start(w_t[:], wk[ct * 128 : (ct + 1) * 128, :])
                wk_sb.append(w_t)
            cos_sb = p_tab.tile([128, t], BF16, tag="cos")
            nc.sync.dma_start(cos_sb[:], cosf[:])
            sin_sb = p_tab.tile([128, t], BF16, tag="sin")
            nc.sync.dma_start(sin_sb[:], sinf[:])
            pswap_sb = p_tab.tile([128, 128], BF16, tag="pswap")
            nc.sync.dma_start(pswap_sb[:], pswap[:])
            blk2_sb = p_tab.tile([128, 2], BF16, tag="blk2")
            nc.sync.dma_start(blk2_sb[:], blk2[:])
            eqb_sb = p_tab.tile([2, 128], BF16, tag="eqb")
            nc.sync.dma_start(eqb_sb[:], eqb[:])
            ekb_sb = p_tab.tile([2, 128], BF16, tag="ekb")
            nc.sync.dma_start(ekb_sb[:], ekb[:])
            mask_sb = p_tab.tile([128, 4 * TB], BF16, tag="mask")
            nc.sync.dma_start(mask_sb[:], maskt[:])
            ones64 = p_tab.tile([1, 64], BF16, tag="ones64")
            nc.vector.memset(ones64[:], 1.0)
            eps_sb = p_tab.tile([128, 1], F32, tag="eps")
            nc.vector.memset(eps_sb[:], EPS)
            for ct in range(KT):
                w_t = p_w.tile([128, GF], BF16, tag="wv")
                nc.sync.dma_start(w_t[:], wv[ct * 128 : (ct + 1) * 128, :])
                wv_sb.append(w_t)
            wo_sb = []
            for mt in range(2 * KT):
                w_t = p_wo.tile([128, C], BF16, tag="wo")
                nc.sync.dma_start(w_t[:], wo[mt * 128 : (mt + 1) * 128, :])
                wo_sb.append(w_t)

            # ---- q/k/v projections, rmsnorm + rope (software-pipelined) -----
            # q/k blocks (proj matmuls) are emitted with their norm/rope
            # chain lagging one block, and one v-tile projection interleaved
            # per block, so PE always has independent matmuls to run while
            # ACT/DVE work through the chain.
            qh_sb = [p_qk.tile([128, t], BF16, tag="qk0", name="qh0"),
                     p_qk.tile([128, t], BF16, tag="qk1", name="qh1")]
            kh_sb = [p_qk.tile([128, t], BF16, tag="qk0", name="kh0"),
                     p_qk.tile([128, t], BF16, tag="qk1", name="kh1")]
            v_sb = []

            def emit_chain(pq, dst, jb, eb_sb):
                # sum of squares per head (via blockdiag-ones matmul)
                sq = p_tmp.tile([128, TB], BF16, tag="sq", name="sq")
                nc.scalar.activation(sq[:], pq[:], AF.Square)
                pss = p_mm.tile([128, TB], F32, tag="mm", name="pss")
                nc.tensor.matmul(pss[0:2, :], blk2_sb[:], sq[:], start=True, stop=True)
                sqm = p_tmp.tile([2, TB], F32, tag="sqm", name="sqm")
                nc.scalar.activation(
                    sqm[:], pss[0:2, :], AF.Sqrt, scale=1.0 / D, bias=eps_sb[0:2, :]
                )
                invf = p_tmp.tile([2, TB], F32, tag="invf", name="invf")
                nc.vector.reciprocal_approx_fast(out=invf[:], in_=sqm[:])
                inv = p_tmp.tile([2, TB], BF16, tag="inv", name="inv")
                nc.vector.tensor_copy(inv[:], invf[:])
                # broadcast inv over the 64 rows of each head (* norm w)
                pinvb = p_mm.tile([128, TB], F32, tag="mm", name="pinvb")
                nc.tensor.matmul(pinvb[:], eb_sb[:], inv[:], start=True, stop=True)
                invb = p_tmp.tile([128, TB], BF16, tag="invb", name="invb")
                nc.vector.tensor_copy(invb[:], pinvb[:])
                qn = p_tmp.tile([128, TB], BF16, tag="qn", name="qn")
                nc.vector.tensor_mul(qn[:], pq[:], invb[:])
                # rope: dst = qn*cos + swap(qn)*sin_signed
                pqs = p_mm.tile([128, TB], F32, tag="mm", name="pqs")
                nc.tensor.matmul(pqs[:], pswap_sb[:], qn[:], start=True, stop=True)
                t1 = p_tmp.tile([128, TB], BF16, tag="t1", name="t1")
                nc.vector.tensor_mul(t1[:], qn[:], cos_sb[:, jb])
                t2 = p_tmp.tile([128, TB], BF16, tag="t2", name="t2")
                nc.vector.tensor_mul(t2[:], pqs[:], sin_sb[:, jb])
                nc.vector.tensor_add(dst[:, jb], t1[:], t2[:])

            def emit_v(tt):
                pv = p_mm.tile([128, TB], F32, tag="mm", name="pv")
                for ct in range(KT):
                    nc.tensor.matmul(
                        pv[:, 0:GF],
                        xt_sb[ct][:, tt * 128 : (tt + 1) * 128],
                        wv_sb[ct][:],
                        start=(ct == 0),
                        stop=(ct == KT - 1),
                    )
                v_t = p_v.tile([128, GH * (D + 1)], BF16, tag="v", name="v_t")
                vsrc = pv[:, 0:GF].rearrange("p (h d) -> p h d", h=GH)
                vdst = v_t[:].rearrange("p (h d) -> p h d", h=GH, d=D + 1)
                nc.vector.tensor_copy(vdst[:, :, 0:D], vsrc)
                nc.vector.memset(vdst[:, :, D : D + 1], 1.0)
                v_sb.append(v_t)

            # Weight-stationary proj: for each (tensor, mt, ct) the weight
            # tile is loaded once and streamed against all ntb token blocks
            # (4 live psum accumulators), keeping the PE array duty high.
            vb = 0
            for w_sb, eb_sb, dst_tiles in (
                (wq_sb, eqb_sb, qh_sb),
                (wk_sb, ekb_sb, kh_sb),
            ):
                for mt in range(2):
                    pqs_j = [
                        p_oacc.tile([128, TB], F32, tag="oacc", name=f"pq{j}")
                        for j in range(ntb)
                    ]
                    for ct in range(KT):
                        for j in range(ntb):
                            nc.tensor.matmul(
                                pqs_j[j][:],
                                w_sb[ct][:, mt * 128 : (mt + 1) * 128],
                                xt_sb[ct][:, j * TB : (j + 1) * TB],
                                start=(ct == 0),
                                stop=(ct == KT - 1),
                            )
                    for j in range(ntb):
                        emit_chain(
                            pqs_j[j], dst_tiles[mt], slice(j * TB, (j + 1) * TB), eb_sb
                        )
                        if vb < ntt:
                            emit_v(vb)
                            vb += 1
                        if vb < ntt:
                            emit_v(vb)
                            vb += 1

            # ---- attention + A2A exchange + o_proj --------------------------
            yg_sb = [None] * (2 * KT)
            bounce_in = []
            bounce_out = []
            for hp in range(2):  # head pairs (2 heads each)
                y_t = p_y.tile([128, t], BF16, tag="y")
                for jp in range(max(1, ntb // 2)):
                    js = [j for j in (2 * jp, 2 * jp + 1) if j < ntb]
                    jbs = {j: slice(j * TB, (j + 1) * TB) for j in js}
                    po = {
                        j: [
                            p_oacc.tile(
                                [D + 1, TB], F32, tag="oacc", name=f"po{j}_{i}"
                            )
                            for i in range(2)
                        ]
                        for j in js
                    }

                    def emit_norm(j):
                        rec = []
                        for hl in range(2):
                            dn = p_tmp.tile(
                                [1, TB], F32, tag=f"den{hl}", name=f"dn{hl}"
                            )
                            nc.vector.tensor_copy(dn[:], po[j][hl][64:65, :])
                            rf = p_tmp.tile(
                                [1, TB], F32, tag=f"recf{hl}", name=f"rf{hl}"
                            )
                            nc.vector.reciprocal_approx_fast(out=rf[:], in_=dn[:])
                            rc = p_tmp.tile(
                                [1, TB], BF16, tag=f"rec{hl}", name=f"rc{hl}"
                            )
                            nc.vector.tensor_copy(rc[:], rf[:])
                            rec.append(rc)
                        pr = p_mm.tile([128, TB], F32, tag="mm", name="pr")
                        nc.tensor.matmul(
                            pr[0:64, :], ones64[:], rec[0][:], start=True, stop=True,
                            tile_position=(0, 0),
                        )
                        nc.tensor.matmul(
                            pr[64:128, :], ones64[:], rec[1][:], start=True,
                            stop=True, tile_position=(0, 64),
                        )
                        r_sb = p_tmp.tile([128, TB], BF16, tag="rsb", name="r_sb")
                        nc.vector.tensor_copy(r_sb[:], pr[:])
                        nc.vector.tensor_mul(
                            y_t[0:64, jbs[j]], po[j][0][0:64, :], r_sb[0:64, :]
                        )
                        nc.vector.tensor_mul(
                            y_t[64:128, jbs[j]], po[j][1][0:64, :], r_sb[64:128, :]
                        )

                    def attn_v(tt, valid, pts):
                        done = []
                        for hl in range(2):
                            h = 2 * hp + hl
                            for j in valid:
                                nc.tensor.matmul(
                                    po[j][hl][:],
                                    v_sb[tt][:, h * (D + 1) : (h + 1) * (D + 1)],
                                    pts[(j, hl)][:],
                                    start=(tt == 0),
                                    stop=(tt == 4 * (j + 1) - 1),
                                )
                        for j in valid:
                            if tt == 4 * (j + 1) - 1:
                                done.append(j)
                        for j in done:
                            emit_norm(j)

                    LAG = 5
                    pend = []
                    n_tt = 4 * (js[-1] + 1)
                    for tt in range(n_tt):
                        valid = [j for j in js if tt < 4 * (j + 1)]
                        ps = {}
                        for hl in range(2):
                            hofs = hl * 64
                            for j in valid:
                                p = p_mm.tile(
                                    [128, TB], F32, tag="mm", name=f"ps{hl}_{j}"
                                )
                                nc.tensor.matmul(
                                    p[:],
                                    kh_sb[hp][
                                        hofs : hofs + 64, tt * 128 : (tt + 1) * 128
                                    ],
                                    qh_sb[hp][hofs : hofs + 64, jbs[j]],
                                    start=True,
                                    stop=True,
                                    tile_position=(hofs, 0),
                                )
                                ps[(j, hl)] = p
                        pts = {}
                        for j in valid:
                            r = tt - 4 * j
                            for hl in range(2):
                                pt = p_pt.tile([128, TB], BF16, tag="pt")
                                nc.scalar.activation(
                                    pt[:],
                                    ps[(j, hl)][:],
                                    AF.Exp,
                                    scale=1.0 / np.sqrt(D),
                                )
                                if r >= 0:  # diagonal tile: apply causal mask
                                    # on GPSIMD: pure-SBUF bf16 op, keeps DVE free
                                    nc.gpsimd.tensor_mul(
                                        pt[:],
                                        pt[:],
                                        mask_sb[:, r * TB : (r + 1) * TB],
                                    )
                                pts[(j, hl)] = pt
                        pend.append((tt, valid, pts))
                        if len(pend) > LAG:
                            attn_v(*pend.pop(0))
                    for item in pend:
                        attn_v(*item)

                # exchange: 8-way AllToAll (4-core groups unsupported).
                # Shard s (dest rank s) carries our features(hp) for token
                # block s%4; rank c thus receives its token block c%4 from
                # every rank.  Rows from other-batch ranks are junk -- the
                # host zeroes the matching rows of wo so o_proj ignores them.
                bin_t = p_dram.tile([8 * 128, tsl], BF16, tag=f"bin{hp}")
                bout_t = p_dram.tile([8 * 128, tsl], BF16, tag=f"bout{hp}")
                bounce_in.append(bin_t)
                bounce_out.append(bout_t)
                for s in range(8):
                    i = s % 4
                    nc.gpsimd.dma_start(
                        bin_t[s * 128 : (s + 1) * 128, :],
                        y_t[:, i * tsl : (i + 1) * tsl],
                    )
                nc.gpsimd.collective_compute(
                    "AllToAll",
                    mybir.AluOpType.bypass,
                    ins=[bin_t.opt()],
                    outs=[bout_t.opt()],
                    replica_groups=[[0, 1, 2, 3, 4, 5, 6, 7]],
                )
                for i in range(8):
                    yg_t = p_yg.tile([128, tsl], BF16, tag="yg")
                    nc.sync.dma_start(yg_t[:], bout_t[i * 128 : (i + 1) * 128, :])
                    yg_sb[2 * i + hp] = yg_t

            # o_proj over our token slice: out^T[cout, tsl]
            mt_order = [2 * i for i in range(KT)] + [2 * i + 1 for i in range(KT)]
            for co in range(KT):
                pout = p_mm.tile([128, tsl], F32, tag="mm")
                for n_mt, mt in enumerate(mt_order):
                    nc.tensor.matmul(
                        pout[:, 0:tsl],
                        wo_sb[mt][:, co * 128 : (co + 1) * 128],
                        yg_sb[mt][:],
                        start=(n_mt == 0),
                        stop=(n_mt == 2 * KT - 1),
                    )
                o_sb = p_tmp.tile([128, tsl], F32, tag="osb")
                nc.vector.tensor_copy(o_sb[:], pout[:, 0:tsl])
                nc.sync.dma_start(out[co * 128 : (co + 1) * 128, :], o_sb[:])

    nc.compile()
    return nc


# ---------------------------------------------------------------------------
# host side
# ---------------------------------------------------------------------------


def _rope_tables(t):
    inv_freq = 1.0 / (ROPE_BASE ** (np.arange(0, D, 2, dtype=np.float64) / D))  # [32]
    ang = np.arange(t, dtype=np.float64)[:, None] * inv_freq[None, :]  # [t, 32]
    cos = np.cos(ang).astype(np.float32)
    sin = np.sin(ang).astype(np.float32)
    cosf = np.empty((128, t), np.float32)
    sinf = np.empty((128, t), np.float32)
    for r in range(128):
        d = r % 64
        f = d if d < 32 else d - 32
        cosf[r] = cos[:, f]
        sinf[r] = -sin[:, f] if d < 32 else sin[:, f]
    return cosf, sinf


def _consts(t):
    cosf, sinf = _rope_tables(t)
    pswap = np.zeros((128, 128), np.float32)
    for j in range(128):
        d = j % 64
        i = (j - 32) if d >= 32 else (j + 32)
        pswap[i, j] = 1.0
    blk2 = np.zeros((128, 2), np.float32)
    blk2[0:64, 0] = 1.0
    blk2[64:128, 1] = 1.0
    maskt = np.zeros((128, 4 * TB), np.float32)
    for r in range(4):
        for p in range(128):
            lo = 128 * r + p
            if lo < TB:
                maskt[p, r * TB + lo : (r + 1) * TB] = 1.0
    return cosf, sinf, pswap, blk2, maskt


def _eb(w):
    e = np.zeros((2, 128), np.float32)
    e[0, 0:64] = w[0:64]
    e[1, 64:128] = w[0:64] if len(w) == 64 else w[64:128]
    return e


def _bf(x):
    return np.ascontiguousarray(x).astype(ml_dtypes.bfloat16)


def make_in_maps(x, Wq, Wk, Wv, Wo, qn_w, kn_w, t=T):
    cosf, sinf, pswap, blk2, maskt = _consts(t)
    eq = _eb(qn_w)
    ek = _eb(kn_w)
    common = {
        "cosf": _bf(cosf),
        "sinf": _bf(sinf),
        "pswap": _bf(pswap),
        "blk2": _bf(blk2),
        "eqb": _bf(eq),
        "ekb": _bf(ek),
        "maskt": _bf(maskt),
    }
    in_maps = []
    for c in range(N_CORES):
        b, g = c // 4, c % 4
        fs = slice(GF * g, GF * (g + 1))
        wot = Wo.T  # [c_in, c_out]
        wo_core = np.zeros((2 * C, C), np.float32)
        for i in range(8):
            if i // 4 == b:
                gi = i % 4
                for hp in range(2):
                    u = 256 * i + 128 * hp
                    f0 = GF * gi + 128 * hp
                    wo_core[u : u + 128, :] = wot[f0 : f0 + 128, :]
        in_maps.append(
            dict(
                common,
                xt=_bf(x[b, :t, :].T),
                wq=_bf(Wq[fs, :].T),
                wk=_bf(Wk[fs, :].T),
                wv=_bf(Wv[fs, :].T),
                wo=_bf(wo_core),
            )
        )
    return in_maps


def assemble(results, t=T):
    tsl = t // 4
    out = np.empty((B, t, C), np.float32)
    for c in range(N_CORES):
        b, g = c // 4, c % 4
        out[b, g * tsl : (g + 1) * tsl, :] = results[c]["out"].T
    return out


# -- cached PJRT runner (compile once, reuse across kernel() calls) ---------

_RUNNER = {}


def _get_runner(t=T):
    if t in _RUNNER:
        return _RUNNER[t]
    import jax
    from jax.sharding import Mesh, PartitionSpec
    from jax.experimental.shard_map import shard_map
    from concourse import bass2jax

    nc = build_nc(t)
    bass2jax.install_neuronx_cc_hook()

    partition_name = nc.partition_id_tensor.name if nc.partition_id_tensor else None
    in_names = []
    out_names = []
    out_avals = []
    zero_outs = []
    for alloc in nc.m.functions[0].allocations:
        if not isinstance(alloc, mybir.MemoryLocationSet):
            continue
        name = alloc.memorylocations[0].name
        if alloc.kind == "ExternalInput":
            if name == partition_name:
                continue
            in_names.append(name)
        elif alloc.kind == "ExternalOutput":
            shape = tuple(alloc.tensor_shape)
            dtype = mybir.dt.np(alloc.dtype)
            out_names.append(name)
            out_avals.append(jax.core.ShapedArray(shape, dtype))
            zero_outs.append(np.zeros(shape, dtype))
    n_params = len(in_names)
    all_names = in_names + out_names
    if partition_name is not None:
        all_names = all_names + [partition_name]

    def _body(*args):
        operands = list(args)
        if partition_name is not None:
            operands.append(bass2jax.partition_id_tensor())
        outs = bass2jax._bass_exec_p.bind(
            *operands,
            out_avals=tuple(out_avals),
            in_names=tuple(all_names),
            out_names=tuple(out_names),
            lowering_input_output_aliases=(),
            sim_require_finite=True,
            sim_require_nnan=True,
            nc=nc,
        )
        return tuple(outs)

    devices = jax.devices()[:N_CORES]
    mesh = Mesh(np.asarray(devices), ("core",))
    fn = jax.jit(
        shard_map(
            _body,
            mesh=mesh,
            in_specs=(PartitionSpec("core"),) * (n_params + len(out_names)),
            out_specs=(PartitionSpec("core"),) * len(out_names),
            check_rep=False,
        ),
        keep_unused=True,
    )
    runner = {
        "fn": fn,
        "body": _body,
        "in_names": in_names,
        "out_names": out_names,
        "out_avals": out_avals,
        "zero_outs": zero_outs,
        "jax": jax,
    }
    _RUNNER[t] = runner
    return runner


def run_device(in_maps, t=T):
    r = _get_runner(t)
    concat_in = [
        np.concatenate([np.asarray(m[name]) for m in in_maps], axis=0)
        for name in r["in_names"]
    ]
    concat_zero = [
        np.zeros((N_CORES * z.shape[0], *z.shape[1:]), z.dtype) for z in r["zero_outs"]
    ]
    outs = r["fn"](*concat_in, *concat_zero)
    results = []
    for c in range(N_CORES):
        results.append(
            {
                name: np.asarray(outs[i]).reshape(N_CORES, *r["out_avals"][i].shape)[c]
                for i, name in enumerate(r["out_names"])
            }
        )
    return results


def kernel(x, Wq, Wk, Wv, Wo, qn_w, kn_w):
    x = np.asarray(x, np.float32)
    in_maps = make_in_maps(
        x,
        np.asarray(Wq, np.float32),
        np.asarray(Wk, np.float32),
        np.asarray(Wv, np.float32),
        np.asarray(Wo, np.float32),
        np.asarray(qn_w, np.float32),
        np.asarray(kn_w, np.float32),
    )
    results = run_device(in_maps)
    return assemble(results)



# revision 31
# speedup vs baseline: 1.1971x; 1.1971x over previous
"""Trainium2 Bass kernel for causal self-attention (RoPE + per-head RMSNorm).

Reference computation (B=2, T=2048, C=1024, H=16, D=64):
    q = rope(rmsnorm(x @ Wq.T)); k = rope(rmsnorm(x @ Wk.T)); v = x @ Wv.T
    out = softmax(causal(q k^T / sqrt(D))) v @ Wo.T

Sharding over 8 NeuronCores: core c -> batch b = c//4, head-group g = c%4
(4 heads = 256 features per group).  Feature-major ("transposed") on-chip
layout; scores computed as S^T[tk, tq]; softmax denominator via a ones
column appended to V (attn-V matmul M=65).

v2 restructure vs baseline:
  - scores for the two heads of a head-pair land in one 2-bank PSUM tile
    [128,1024]; ONE exp activation covers both (halves ACT op count).
  - causal shrink: score/AV matmuls and exp skip fully-masked column
    ranges; only the diagonal 128x128 block is masked (DVE, [128,2,128]).
  - per-head norm weight folded into host-built rope tables; inv-rms and
    1/den broadcasts via tiny fp32 matmuls (no bf16 casts).
  - engine balance: exp/square/sqrt on ACT, muls on DVE, rope mul+add on
    GPSIMD, all matmuls PE, dmas on sync.
  - dense PE stream: proj(mt=1) woven into attention(hp=0) steps, o_proj
    wave0 woven into attention(hp=1), AV pass for the second head woven
    into the next query-block's score stream -> HAM stays warm.
  - AllToAll split into 8 pieces (hp x query-block token stripes), fired
    as soon as each block's output is ready; o_proj consumes stripes.
"""

import os
import sys

for _p in ("/opt/trn_rl_repo", "/root/.axon_site/_ro/trn_rl_repo"):
    if os.path.isdir(_p) and _p not in sys.path:
        sys.path.insert(0, _p)

import numpy as np
import ml_dtypes

import concourse.bass as bass
from concourse import bacc
import concourse.tile as tile
import concourse.mybir as mybir

BF16 = mybir.dt.bfloat16
F32 = mybir.dt.float32
AF = mybir.ActivationFunctionType

B, T, C, H, D = 2, 2048, 1024, 16, 64
N_CORES = 8
GH = 4  # heads per core
GF = GH * D  # features per core (256)
TB = 512  # token block (matmul N)
KT = C // 128  # 8 contraction k-tiles
EPS = float(np.finfo(np.float32).eps)
ROPE_BASE = 10000.0
ISD = 1.0 / np.sqrt(D)


def build_nc(t=T):
    ntb = t // TB  # query blocks (4)
    ntt = t // 128  # token 128-tiles (16)
    tsl = t // 4  # per-core o_proj token count (512)
    nst = tsl // ntb  # stripe width (128)

    nc = bacc.Bacc("TRN2", target_bir_lowering=False, debug=False, num_devices=N_CORES)

    xt = nc.dram_tensor("xt", [C, t], BF16, kind="ExternalInput")
    wq = nc.dram_tensor("wq", [C, GF], BF16, kind="ExternalInput")
    wk = nc.dram_tensor("wk", [C, GF], BF16, kind="ExternalInput")
    wv = nc.dram_tensor("wv", [C, GF], BF16, kind="ExternalInput")
    wo = nc.dram_tensor("wo", [C, C], BF16, kind="ExternalInput")
    cosq = nc.dram_tensor("cosq", [128, t], BF16, kind="ExternalInput")
    sinq = nc.dram_tensor("sinq", [128, t], BF16, kind="ExternalInput")
    cosk = nc.dram_tensor("cosk", [128, t], BF16, kind="ExternalInput")
    sink = nc.dram_tensor("sink", [128, t], BF16, kind="ExternalInput")
    pswap = nc.dram_tensor("pswap", [128, 128], BF16, kind="ExternalInput")
    blk2 = nc.dram_tensor("blk2", [128, 2], BF16, kind="ExternalInput")
    e2b = nc.dram_tensor("e2b", [2, 128], BF16, kind="ExternalInput")
    mask2 = nc.dram_tensor("mask2", [128, 2 * 128], BF16, kind="ExternalInput")
    out = nc.dram_tensor("out", [C, tsl], F32, kind="ExternalOutput")

    with tile.TileContext(nc) as tc:
        with (
            nc.allow_low_precision(reason="bf16 compute by design"),
            tc.tile_pool(name="p_xt", bufs=KT) as p_xt,
            tc.tile_pool(name="p_w", bufs=KT) as p_w,
            tc.tile_pool(name="p_wo", bufs=KT) as p_wo,
            tc.tile_pool(name="p_tab", bufs=1) as p_tab,
            tc.tile_pool(name="p_qk", bufs=2) as p_qk,
            tc.tile_pool(name="p_v", bufs=ntt) as p_v,
            tc.tile_pool(name="p_y", bufs=1) as p_y,
            tc.tile_pool(name="p_yg", bufs=KT) as p_yg,
            tc.tile_pool(name="p_pt", bufs=ntt) as p_pt,
            tc.tile_pool(name="p_tmp", bufs=3) as p_tmp,
            tc.tile_pool(name="p_oacc", bufs=KT) as p_oacc,
            tc.tile_pool(name="p_mm2", bufs=2, space="PSUM") as p_mm2,
            tc.tile_pool(name="p_po", bufs=2, space="PSUM") as p_po,
            tc.tile_pool(name="p_wk", bufs=2, space="PSUM") as p_wk,
            tc.tile_pool(name="p_dram", bufs=2, space="DRAM") as p_dram,
        ):
            # ---- input loads (DMA order = priority order) --------------------
            wq_sb = []
            wk_sb = []
            wv_sb = []
            xt_sb = []
            for ct in range(KT):
                w_t = p_w.tile([128, GF], BF16, tag="wq", name=f"wqt{ct}")
                nc.sync.dma_start(w_t[:], wq[ct * 128 : (ct + 1) * 128, :])
                wq_sb.append(w_t)
            # x loaded in two column halves so the first projection blocks can
            # start before the whole 4 MiB lands.
            for ct in range(KT):
                x_t = p_xt.tile([128, t], BF16, tag="xt", name=f"xtt{ct}")
                nc.sync.dma_start(
                    x_t[:, 0 : t // 2], xt[ct * 128 : (ct + 1) * 128, 0 : t // 2]
                )
                xt_sb.append(x_t)
            for ct in range(KT):
                w_t = p_w.tile([128, GF], BF16, tag="wk", name=f"wkt{ct}")
                nc.sync.dma_start(w_t[:], wk[ct * 128 : (ct + 1) * 128, :])
                wk_sb.append(w_t)
            cosq_sb = p_tab.tile([128, t], BF16, tag="cosq")
            nc.sync.dma_start(cosq_sb[:], cosq[:])
            sinq_sb = p_tab.tile([128, t], BF16, tag="sinq")
            nc.sync.dma_start(sinq_sb[:], sinq[:])
            pswap_sb = p_tab.tile([128, 128], BF16, tag="pswap")
            nc.sync.dma_start(pswap_sb[:], pswap[:])
            blk2_sb = p_tab.tile([128, 2], BF16, tag="blk2")
            nc.sync.dma_start(blk2_sb[:], blk2[:])
            e2b_sb = p_tab.tile([2, 128], BF16, tag="e2b")
            nc.sync.dma_start(e2b_sb[:], e2b[:])
            for ct in range(KT):
                nc.sync.dma_start(
                    xt_sb[ct][:, t // 2 : t], xt[ct * 128 : (ct + 1) * 128, t // 2 : t]
                )
            for ct in range(KT):
                w_t = p_w.tile([128, GF], BF16, tag="wv", name=f"wvt{ct}")
                nc.sync.dma_start(w_t[:], wv[ct * 128 : (ct + 1) * 128, :])
                wv_sb.append(w_t)
            cosk_sb = p_tab.tile([128, t], BF16, tag="cosk")
            nc.sync.dma_start(cosk_sb[:], cosk[:])
            sink_sb = p_tab.tile([128, t], BF16, tag="sink")
            nc.sync.dma_start(sink_sb[:], sink[:])
            mask2_sb = p_tab.tile([128, 2 * 128], BF16, tag="mask2")
            nc.sync.dma_start(mask2_sb[:], mask2[:])
            eps_sb = p_tab.tile([128, 1], F32, tag="eps")
            nc.vector.memset(eps_sb[:], EPS)
            wo_sb = []
            for mt in range(KT):
                w_t = p_wo.tile([128, C], BF16, tag="wo", name=f"wot{mt}")
                nc.sync.dma_start(w_t[:], wo[mt * 128 : (mt + 1) * 128, :])
                wo_sb.append(w_t)

            # register with 4*(batch group) = 4*(rank//4), for dynamic slot
            # selection out of the AllToAll result (only same-batch slots
            # carry our batch's attention output).
            rb_reg = nc.sync.alloc_register()
            nc.sync.cc_rank_ld(rb_reg, replica_groups=[[0, 1, 2, 3, 4, 5, 6, 7]])
            nc.sync.reg_div(rb_reg, rb_reg, 4)
            nc.sync.reg_mul(rb_reg, rb_reg, 4)
            rb4s = nc.sync.snap(rb_reg, min_val=0, max_val=4)

            qh_sb = [p_qk.tile([128, t], BF16, tag="qk0", name="qh0"),
                     p_qk.tile([128, t], BF16, tag="qk1", name="qh1")]
            kh_sb = [p_qk.tile([128, t], BF16, tag="qk0", name="kh0"),
                     p_qk.tile([128, t], BF16, tag="qk1", name="kh1")]
            v_sb = []
            y_sb = [p_y.tile([128, t], BF16, tag="y0", name="y0"),
                    p_y.tile([128, t], BF16, tag="y1", name="y1")]
            yg_sb = [None] * KT  # index m = 2*gi + hp
            for i in range(KT):
                yg_sb[i] = p_yg.tile([128, tsl], BF16, tag="yg", name=f"yg{i}")
            oacc_sb = []
            for co in range(KT):
                o_t = p_oacc.tile([128, tsl], BF16, tag="oacc", name=f"oac{co}")
                oacc_sb.append(o_t)

            # ---- proj block: 8 acc MMs + rmsnorm/rope chain ------------------
            def emit_proj(which, mt, j):
                jb = slice(j * TB, (j + 1) * TB)
                w_sb = wq_sb if which == "q" else wk_sb
                cos_sb = cosq_sb if which == "q" else cosk_sb
                sin_sb = sinq_sb if which == "q" else sink_sb
                dst = (qh_sb if which == "q" else kh_sb)[mt]
                acc = p_wk.tile([128, TB], F32, tag="w", name="acc")
                for ct in range(KT):
                    nc.tensor.matmul(
                        acc[:],
                        w_sb[ct][:, mt * 128 : (mt + 1) * 128],
                        xt_sb[ct][:, jb],
                        start=(ct == 0),
                        stop=(ct == KT - 1),
                    )
                # rmsnorm deferred past rope: inv is constant within a head's
                # 64 rows, so it commutes with the half-swap rotation.
                pqb = p_tmp.tile([128, TB], BF16, tag="qn", name="pqb")
                nc.vector.tensor_copy(pqb[:], acc[:])
                sq = p_tmp.tile([128, TB], BF16, tag="sq", name="sq", bufs=2)
                nc.scalar.activation(sq[:], acc[:], AF.Square)
                # pw holds the [2,TB] sum-of-squares first, then (after the
                # sqrt consumed it) the [128,TB] broadcast inv -- sequential
                # lifetimes in one PSUM bank.
                pw = p_wk.tile([128, TB], F32, tag="w", name="pw")
                nc.tensor.matmul(pw[0:2, :], blk2_sb[:], sq[:], start=True, stop=True)
                sqm = p_tmp.tile([2, TB], F32, tag="sqm", name="sqm", bufs=2)
                nc.scalar.activation(
                    sqm[:], pw[0:2, :], AF.Sqrt, scale=1.0 / D, bias=eps_sb[0:2, :]
                )
                inv = p_tmp.tile([2, TB], F32, tag="inv", name="inv", bufs=2)
                nc.vector.reciprocal_approx_fast(out=inv[:], in_=sqm[:])
                invb = p_tmp.tile([2, TB], BF16, tag="invb", name="invb", bufs=2)
                nc.vector.tensor_copy(invb[:], inv[:])
                nc.tensor.matmul(pw[:], e2b_sb[:], invb[:], start=True, stop=True)
                pqs = p_wk.tile([128, TB], F32, tag="w", name="pqs")
                nc.tensor.matmul(pqs[:], pswap_sb[:], pqb[:], start=True, stop=True)
                t1 = p_tmp.tile([128, TB], BF16, tag="t1", name="t1")
                nc.gpsimd.tensor_mul(t1[:], pqb[:], cos_sb[:, jb])
                t2 = p_tmp.tile([128, TB], BF16, tag="t2", name="t2")
                nc.vector.tensor_mul(t2[:], pqs[:], sin_sb[:, jb])
                rsum = p_tmp.tile([128, TB], BF16, tag="rs", name="rsum")
                nc.gpsimd.tensor_add(rsum[:], t1[:], t2[:])
                nc.vector.tensor_mul(dst[:, jb], rsum[:], pw[:])

            # ---- v tile: token-major projection + ones column ----------------
            def emit_v(tt):
                pv = p_wk.tile([128, TB], F32, tag="w", name="pv")
                for ct in range(KT):
                    nc.tensor.matmul(
                        pv[:, 0:GF],
                        xt_sb[ct][:, tt * 128 : (tt + 1) * 128],
                        wv_sb[ct][:],
                        start=(ct == 0),
                        stop=(ct == KT - 1),
                    )
                v_t = p_v.tile([128, GH * (D + 1)], BF16, tag="v", name="v_t")
                vsrc = pv[:, 0:GF].rearrange("p (h d) -> p h d", h=GH)
                vdst = v_t[:].rearrange("p (h d) -> p h d", h=GH, d=D + 1)
                nc.vector.tensor_copy(vdst[:, :, 0:D], vsrc)
                nc.vector.memset(vdst[:, :, D : D + 1], 1.0)
                v_sb.append(v_t)

            # ---- attention ---------------------------------------------------
            # state for the woven AV passes: per (hp, j) the list of pt tiles
            pt_tiles = {}

            def emit_S_exp(hp, j, tt):
                """scores pair matmul + exp (+ diagonal mask)."""
                jb0 = j * TB
                r = tt - 4 * j  # >=0 on diagonal tiles
                c0 = 128 * r if r >= 0 else 0
                psp = p_mm2.tile([128, 2 * TB], F32, tag="mm2", name="psp")
                for hl in range(2):
                    hofs = hl * 64
                    nc.tensor.matmul(
                        psp[:, hl * TB + c0 : (hl + 1) * TB],
                        kh_sb[hp][hofs : hofs + 64, tt * 128 : (tt + 1) * 128],
                        qh_sb[hp][hofs : hofs + 64, jb0 + c0 : jb0 + TB],
                        start=True,
                        stop=True,
                        tile_position=(hofs, 0),
                    )
                pt = p_pt.tile([128, 2 * TB], BF16, tag="pt", name="pt")
                pt3 = pt[:].rearrange("p (h c) -> p h c", h=2)
                ps3 = psp[:].rearrange("p (h c) -> p h c", h=2)
                nc.scalar.activation(
                    pt3[:, :, c0:TB], ps3[:, :, c0:TB], AF.Exp, scale=ISD
                )
                if r >= 0:
                    m3 = mask2_sb[:].rearrange("p (h c) -> p h c", h=2)
                    nc.gpsimd.tensor_mul(
                        pt3[:, :, c0 : c0 + 128], pt3[:, :, c0 : c0 + 128], m3
                    )
                pt_tiles[(hp, j)].append((tt, c0, pt))

            def emit_AV(hp, j, hl, po, tt, c0, pt):
                h = 2 * hp + hl
                nc.tensor.matmul(
                    po[0 : D + 1, c0:TB],
                    v_sb[tt][:, h * (D + 1) : (h + 1) * (D + 1)],
                    pt[:, hl * TB + c0 : (hl + 1) * TB],
                    start=(tt == 0),
                    stop=(tt == 4 * (j + 1) - 1),
                )

            def emit_div(hp, j, hl, po):
                jb = slice(j * TB, (j + 1) * TB)
                dn = p_tmp.tile([1, TB], F32, tag="dn", name="dn", bufs=2)
                nc.vector.tensor_copy(dn[:], po[D : D + 1, :])
                rec = p_tmp.tile([1, TB], F32, tag="rec", name="rec", bufs=2)
                nc.vector.reciprocal_approx_fast(out=rec[:], in_=dn[:])
                rc = p_tmp.tile([1, TB], BF16, tag="rc", name="rc", bufs=2)
                nc.vector.tensor_copy(rc[:], rec[:])
                pr = p_wk.tile([128, TB], F32, tag="w", name="pr")
                nc.tensor.matmul(
                    pr[0:64, :], e2b_sb[0:1, 0:64], rc[:], start=True, stop=True
                )
                prb = p_tmp.tile([64, TB], BF16, tag="prb", name="prb", bufs=2)
                nc.vector.tensor_copy(prb[:], pr[0:64, :])
                nc.vector.tensor_mul(
                    y_sb[hp][hl * 64 : hl * 64 + 64, jb], po[0:D, :], prb[:]
                )

            # ---- A2A piece (hp, j): exchange token stripes -------------------
            cc_bufs = []

            def emit_piece(hp, j):
                bin_t = p_dram.tile([8 * 128, nst], BF16, tag="bin", name=f"bi{hp}{j}")
                bout_t = p_dram.tile([8, 128, nst], BF16, tag="bout", name=f"bo{hp}{j}")
                cc_bufs.append((bin_t, bout_t))
                for s in range(8):
                    g = s % 4
                    nc.gpsimd.dma_start(
                        bin_t[s * 128 : (s + 1) * 128, :],
                        y_sb[hp][:, j * TB + g * nst : j * TB + (g + 1) * nst],
                    )
                nc.gpsimd.collective_compute(
                    "AllToAll",
                    mybir.AluOpType.bypass,
                    ins=[bin_t.opt()],
                    outs=[bout_t.opt()],
                    replica_groups=[[0, 1, 2, 3, 4, 5, 6, 7]],
                )
                # dynamic slot select: only the 4 same-batch source ranks
                # carry our batch; their slot index is rb4s + gi.
                for gi in range(4):
                    nc.sync.dma_start(
                        yg_sb[2 * gi + hp][:, j * nst : (j + 1) * nst],
                        bout_t[rb4s + gi],
                    )

            # ---- o_proj wave: one column tile over 8 slots of one hp ---------
            def emit_wave(hp, co):
                pout = p_wk.tile([128, tsl], F32, tag="w", name="pout")
                for gi in range(4):
                    m = 2 * gi + hp
                    nc.tensor.matmul(
                        pout[:, 0:tsl],
                        wo_sb[m][:, co * 128 : (co + 1) * 128],
                        yg_sb[m][:],
                        start=(gi == 0),
                        stop=(gi == 3),
                    )
                if hp == 0:
                    nc.vector.tensor_copy(oacc_sb[co][:], pout[:, 0:tsl])
                else:
                    ofin = p_tmp.tile([128, tsl], F32, tag="ofin", name="ofin", bufs=2)
                    nc.vector.tensor_add(ofin[:], pout[:, 0:tsl], oacc_sb[co][:])
                    nc.sync.dma_start(out[co * 128 : (co + 1) * 128, :], ofin[:])

            # =================== emission schedule ===========================
            # Phase A: proj mt=0 (+ all v tiles)
            for j in range(ntb):
                emit_proj("q", 0, j)
                emit_v(4 * j + 0)
                emit_v(4 * j + 1)
                emit_proj("k", 0, j)
                emit_v(4 * j + 2)
                emit_v(4 * j + 3)

            # Phase B/C: attention hp with filler quanta woven in
            AV_LAG = 3

            def run_attn(hp, fillers, start_after=0, tail_burst=None):
                """fillers: closures popped gradually between steps (after
                step index start_after).  Returns the last j's deferred hl=1
                burst for the caller to weave into what follows."""
                fill_i = [0]
                total_steps = sum(4 * (j + 1) for j in range(ntb))
                per_step = len(fillers) / max(total_steps - start_after, 1)
                credit = [0.0]
                step_n = [0]
                tail = list(tail_burst or [])

                def step_fill():
                    step_n[0] += 1
                    if step_n[0] <= start_after:
                        return
                    credit[0] += per_step
                    while fill_i[0] < len(fillers) and credit[0] >= 1.0:
                        fillers[fill_i[0]]()
                        fill_i[0] += 1
                        credit[0] -= 1.0

                prev_burst = []  # deferred hl=1 AV work of previous j
                for j in range(ntb):
                    pt_tiles[(hp, j)] = []
                    po0 = p_po.tile([D + 1, TB], F32, tag="po", name="po0")
                    n_tt = 4 * (j + 1)
                    pend = []
                    burst = list(prev_burst)
                    for tt in range(n_tt):
                        # weave deferred work BEFORE the pt alloc in emit_S_exp
                        # so displaced pt tiles' readers are already emitted
                        for _ in range(3):
                            if tail:
                                tail.pop(0)()
                        for _ in range(3):
                            if burst:
                                burst.pop(0)()
                        emit_S_exp(hp, j, tt)
                        pend.append(pt_tiles[(hp, j)][-1])
                        if len(pend) > AV_LAG:
                            ttx, c0x, ptx = pend.pop(0)
                            emit_AV(hp, j, 0, po0, ttx, c0x, ptx)
                        step_fill()
                    for item in burst:
                        item()
                    for ttx, c0x, ptx in pend:
                        emit_AV(hp, j, 0, po0, ttx, c0x, ptx)
                    emit_div(hp, j, 0, po0)
                    # hl=1 AV pass is deferred into the next j's score stream
                    po1 = p_po.tile([D + 1, TB], F32, tag="po", name="po1")

                    def make_burst(hp=hp, j=j, po1=po1):
                        items = []
                        for ttx, c0x, ptx in pt_tiles[(hp, j)]:
                            items.append(
                                lambda ttx=ttx, c0x=c0x, ptx=ptx: emit_AV(
                                    hp, j, 1, po1, ttx, c0x, ptx
                                )
                            )
                        items.append(lambda: emit_div(hp, j, 1, po1))
                        items.append(lambda: emit_piece(hp, j))
                        return items

                    prev_burst = make_burst()
                while fill_i[0] < len(fillers):
                    fillers[fill_i[0]]()
                    fill_i[0] += 1
                for item in tail:
                    item()
                return prev_burst

            projB = []
            for j in range(ntb):
                projB.append(lambda j=j: emit_proj("q", 1, j))
                projB.append(lambda j=j: emit_proj("k", 1, j))
            burst0 = run_attn(0, projB)

            waveC = [lambda co=co: emit_wave(0, co) for co in range(KT)]
            burst1 = run_attn(1, waveC, start_after=10, tail_burst=burst0)

            # Phase D: drain last burst, final o_proj wave (hp=1) + output
            for item in burst1:
                item()
            for co in range(KT):
                emit_wave(1, co)

    nc.compile()
    return nc


# ---------------------------------------------------------------------------
# host side
# ---------------------------------------------------------------------------


def _rope_tables(t, w):
    """[128, t] cos/sin tables for one tensor, with the per-dim norm weight
    folded in and the rope sign folded into sin."""
    inv_freq = 1.0 / (ROPE_BASE ** (np.arange(0, D, 2, dtype=np.float64) / D))  # [32]
    ang = np.arange(t, dtype=np.float64)[:, None] * inv_freq[None, :]  # [t, 32]
    cos = np.cos(ang).astype(np.float32)
    sin = np.sin(ang).astype(np.float32)
    cosf = np.empty((128, t), np.float32)
    sinf = np.empty((128, t), np.float32)
    for r in range(128):
        d = r % 64
        f = d if d < 32 else d - 32
        # the sin term multiplies the PARTNER dim's (pre-norm-weight) value,
        # so it carries the output dim's weight w[d] -- same as cos.  Note
        # norm weight applies per OUTPUT dim before rotation in the
        # reference; factoring: out[d] = w[d]*(x[d]inv)cos - w[d]*(x[p]inv)sin
        # is wrong in general -- reference scales x[p] by w[p].  We fold
        # w into both tables using the output row's weight only when
        # w[d] == w[partner]; for safety fold w[d] into cos and w[partner]
        # into sin.
        p = d + 32 if d < 32 else d - 32
        cosf[r] = cos[:, f] * w[d]
        sinf[r] = (-sin[:, f] if d < 32 else sin[:, f]) * w[p]
    return cosf, sinf


def _consts():
    pswap = np.zeros((128, 128), np.float32)
    for j in range(128):
        d = j % 64
        i = (j - 32) if d >= 32 else (j + 32)
        pswap[i, j] = 1.0
    blk2 = np.zeros((128, 2), np.float32)
    blk2[0:64, 0] = 1.0
    blk2[64:128, 1] = 1.0
    e2b = np.zeros((2, 128), np.float32)
    e2b[0, 0:64] = 1.0
    e2b[1, 64:128] = 1.0
    mask2 = np.zeros((128, 256), np.float32)
    for p in range(128):
        mask2[p, p:128] = 1.0
        mask2[p, 128 + p : 256] = 1.0
    return pswap, blk2, e2b, mask2


def _bf(x):
    return np.ascontiguousarray(x).astype(ml_dtypes.bfloat16)


def make_in_maps(x, Wq, Wk, Wv, Wo, qn_w, kn_w, t=T):
    pswap, blk2, e2b, mask2 = _consts()
    cosq, sinq = _rope_tables(t, qn_w)
    cosk, sink = _rope_tables(t, kn_w)
    common = {
        "cosq": _bf(cosq),
        "sinq": _bf(sinq),
        "cosk": _bf(cosk),
        "sink": _bf(sink),
        "pswap": _bf(pswap),
        "blk2": _bf(blk2),
        "e2b": _bf(e2b),
        "mask2": _bf(mask2),
    }
    wot = np.ascontiguousarray(Wo.T)  # [c_in, c_out]
    # rows ordered by slot m = 2*gi + hp -> features [GF*gi + 128*hp : +128]
    wo_core = np.zeros((C, C), np.float32)
    for gi in range(4):
        for hp in range(2):
            u = 128 * (2 * gi + hp)
            f0 = GF * gi + 128 * hp
            wo_core[u : u + 128, :] = wot[f0 : f0 + 128, :]
    in_maps = []
    for c in range(N_CORES):
        b, g = c // 4, c % 4
        fs = slice(GF * g, GF * (g + 1))
        in_maps.append(
            dict(
                common,
                xt=_bf(x[b, :t, :].T),
                wq=_bf(Wq[fs, :].T),
                wk=_bf(Wk[fs, :].T),
                wv=_bf(Wv[fs, :].T),
                wo=_bf(wo_core),
            )
        )
    return in_maps


def assemble(results, t=T):
    ntb = t // TB
    nst = (t // 4) // ntb
    out = np.empty((B, t, C), np.float32)
    for c in range(N_CORES):
        b, g = c // 4, c % 4
        r = results[c]["out"]  # [C, tsl], cols = ntb stripes of width nst
        for j in range(ntb):
            tok0 = j * TB + g * nst
            out[b, tok0 : tok0 + nst, :] = r[:, j * nst : (j + 1) * nst].T
    return out


# -- cached PJRT runner (compile once, reuse across kernel() calls) ---------

_RUNNER = {}


def _get_runner(t=T):
    if t in _RUNNER:
        return _RUNNER[t]
    import jax
    from jax.sharding import Mesh, PartitionSpec
    from jax.experimental.shard_map import shard_map
    from concourse import bass2jax

    nc = build_nc(t)
    bass2jax.install_neuronx_cc_hook()

    partition_name = nc.partition_id_tensor.name if nc.partition_id_tensor else None
    in_names = []
    out_names = []
    out_avals = []
    zero_outs = []
    for alloc in nc.m.functions[0].allocations:
        if not isinstance(alloc, mybir.MemoryLocationSet):
            continue
        name = alloc.memorylocations[0].name
        if alloc.kind == "ExternalInput":
            if name == partition_name:
                continue
            in_names.append(name)
        elif alloc.kind == "ExternalOutput":
            shape = tuple(alloc.tensor_shape)
            dtype = mybir.dt.np(alloc.dtype)
            out_names.append(name)
            out_avals.append(jax.core.ShapedArray(shape, dtype))
            zero_outs.append(np.zeros(shape, dtype))
    n_params = len(in_names)
    all_names = in_names + out_names
    if partition_name is not None:
        all_names = all_names + [partition_name]

    def _body(*args):
        operands = list(args)
        if partition_name is not None:
            operands.append(bass2jax.partition_id_tensor())
        outs = bass2jax._bass_exec_p.bind(
            *operands,
            out_avals=tuple(out_avals),
            in_names=tuple(all_names),
            out_names=tuple(out_names),
            lowering_input_output_aliases=(),
            sim_require_finite=True,
            sim_require_nnan=True,
            nc=nc,
        )
        return tuple(outs)

    devices = jax.devices()[:N_CORES]
    mesh = Mesh(np.asarray(devices), ("core",))
    fn = jax.jit(
        shard_map(
            _body,
            mesh=mesh,
            in_specs=(PartitionSpec("core"),) * (n_params + len(out_names)),
            out_specs=(PartitionSpec("core"),) * len(out_names),
            check_rep=False,
        ),
        keep_unused=True,
    )
    runner = {
        "fn": fn,
        "body": _body,
        "in_names": in_names,
        "out_names": out_names,
        "out_avals": out_avals,
        "zero_outs": zero_outs,
        "jax": jax,
    }
    _RUNNER[t] = runner
    return runner


def run_device(in_maps, t=T):
    r = _get_runner(t)
    concat_in = [
        np.concatenate([np.asarray(m[name]) for m in in_maps], axis=0)
        for name in r["in_names"]
    ]
    concat_zero = [
        np.zeros((N_CORES * z.shape[0], *z.shape[1:]), z.dtype) for z in r["zero_outs"]
    ]
    outs = r["fn"](*concat_in, *concat_zero)
    results = []
    for c in range(N_CORES):
        results.append(
            {
                name: np.asarray(outs[i]).reshape(N_CORES, *r["out_avals"][i].shape)[c]
                for i, name in enumerate(r["out_names"])
            }
        )
    return results


def kernel(x, Wq, Wk, Wv, Wo, qn_w, kn_w):
    x = np.asarray(x, np.float32)
    in_maps = make_in_maps(
        x,
        np.asarray(Wq, np.float32),
        np.asarray(Wk, np.float32),
        np.asarray(Wv, np.float32),
        np.asarray(Wo, np.float32),
        np.asarray(qn_w, np.float32),
        np.asarray(kn_w, np.float32),
    )
    results = run_device(in_maps)
    return assemble(results)
